# revision 11
# baseline (speedup 1.0000x reference)
"""Trainium2 Bass kernel for GPT2-style attention (nn_Attention_71030169141804).

Computation (faithful to the reference's direct-reshape head split):
  qkv = hidden @ w_attn + b_attn ; q,k,v = split(qkv)
  q/k/v heads = flat-memory reshape (bs, 12, 2048, 64)   <- NOT the standard
     split-heads transpose: head h covers a contiguous 131072-element block of
     the flattened per-batch (seq*768) q/k/v memory, i.e. heads {3g,3g+1,3g+2}
     derive exactly from hidden rows [512g, 512(g+1)).
  causal softmax(qk/8 + mask), @v, merge heads, @ w_proj + b_proj.

Sharding: 8 cores = (batch b in 0..1) x (row-group g in 0..3). Each core:
  - QKV-projects its 512 hidden rows (bias folded in via augmented ones-row),
  - runs attention for its 3 heads (full 2048 seq each),
  - computes the partial output projection with its 192 w_proj rows.
Host sums the 4 partials per batch and adds b_proj. No collectives.

Device dataflow per core (all matmuls bf16, fp32 accumulation):
  P1: qkvT tiles = xt_aug.T @ w_aug -> psum -> sbuf bf16 -> DRAM scratch
      (q/k written into a per-head zero-padded [2048,128] layout).
  P2: per head: QT/KT via DMA-transpose ([2048,128] -> [128,2048]); rows 64+
      carry the attention-mask row trick (KT row64 = 8*mask, QT row64 = 1) so
      S^T = KT.T@QT includes the additive mask exactly; exp fused with the
      1/sqrt(64) scale on ACT; causal masking via affine_select on diagonal
      tiles; PV via V augmented with a ones column so psum row 64 accumulates
      the softmax denominators; normalize A^T with a reciprocal broadcast.
  P3: partial_out = sum_h A^T_h.T @ w_proj[h-rows] -> fp32 -> DRAM.
"""

import numpy as np
import ml_dtypes

BF16 = ml_dtypes.bfloat16

_CACHED_NC = None
_LAST_RESULTS = None


def _build_program():
    from contextlib import ExitStack

    import concourse.bass as bass
    import concourse.tile as tile
    from concourse import bacc, mybir

    dt = mybir.dt
    f32 = dt.float32
    bf16 = dt.bfloat16
    AF = mybir.ActivationFunctionType
    ALU = mybir.AluOpType

    nc = bacc.Bacc("TRN2")
    xt_d = nc.dram_tensor("xt", [896, 512], bf16, kind="ExternalInput")
    wqkv_d = nc.dram_tensor("wqkv", [896, 2304], bf16, kind="ExternalInput")
    wp_d = nc.dram_tensor("wp", [192, 768], bf16, kind="ExternalInput")
    amask_d = nc.dram_tensor("amask", [1, 2048], bf16, kind="ExternalInput")
    out_d = nc.dram_tensor("out", [2048, 768], f32, kind="ExternalOutput")

    xt, wqkv, wp, amask, out = (
        xt_d.ap(),
        wqkv_d.ap(),
        wp_d.ap(),
        amask_d.ap(),
        out_d.ap(),
    )

    with tile.TileContext(nc) as tc, ExitStack() as ctx:
        singles = ctx.enter_context(tc.tile_pool(name="singles", bufs=1))
        dram = ctx.enter_context(tc.tile_pool(name="dram", bufs=1, space="DRAM"))

        w_sb = []
        for kt in range(7):
            t = singles.tile([128, 2304], bf16, name=f"w_sb{kt}", tag=f"w_sb{kt}")
            nc.sync.dma_start(out=t, in_=wqkv[128 * kt : 128 * (kt + 1), :])
            w_sb.append(t)
        xt_sb = []
        for kt in range(7):
            t = singles.tile([128, 512], bf16, name=f"xt_sb{kt}", tag=f"xt_sb{kt}")
            nc.sync.dma_start(out=t, in_=xt[128 * kt : 128 * (kt + 1), :])
            xt_sb.append(t)
        wp_sb = []
        for h in range(3):
            t = singles.tile([64, 768], bf16, name=f"wp_sb{h}", tag=f"wp_sb{h}")
            nc.sync.dma_start(out=t, in_=wp[64 * h : 64 * (h + 1), :])
            wp_sb.append(t)
        ones_col = singles.tile([1, 64], f32, name="ones_col", tag="ones_col")
        nc.gpsimd.memset(ones_col, 1.0)
        AT = []
        for h in range(3):
            t = singles.tile([64, 2048], bf16, name=f"at{h}", tag=f"at{h}")
            AT.append(t)

        q_pad = dram.tile([6144, 128], bf16, name="q_pad", tag="q_pad")
        k_pad = dram.tile([6144, 128], bf16, name="k_pad", tag="k_pad")
        v_scr = dram.tile([512, 768], bf16, name="v_scr", tag="v_scr")

        # ---------------- Phase 1: QKV projection ----------------
        CCH = [(0, 512), (512, 512), (1024, 512), (1536, 512), (2048, 256)]
        with tc.tile_pool(name="p1ps", bufs=3, space="PSUM") as p1ps, tc.tile_pool(
            name="qkvrow", bufs=4
        ) as qkvp:
            for r in range(4):
                qkv_row = qkvp.tile([128, 2304], bf16, name="qkv_row", tag="qkv_row")
                for ci, (c0, cw) in enumerate(CCH):
                    ps = p1ps.tile([128, 512], f32, name="p1t", tag="p1t")
                    for kt in range(7):
                        nc.tensor.matmul(
                            ps[:, :cw],
                            xt_sb[kt][:, 128 * r : 128 * (r + 1)],
                            w_sb[kt][:, c0 : c0 + cw],
                            start=(kt == 0),
                            stop=(kt == 6),
                        )
                    # producer on ACT so the scratch DMAs (also issued from
                    # ACT) need no RAW wait: DMA pseudo-instructions have a
                    # single wait slot (walrus setupSyncWait limit).
                    nc.scalar.copy(out=qkv_row[:, c0 : c0 + cw], in_=ps[:, :cw])
                # q/k sections -> padded per-head layout: flat element
                # (128r+p)*768 + 64*cb + ci2 lands at row 12*(128r+p)+cb,
                # col ci2 of the [6144,128] scratch (cols 64-127 stay zero).
                for sec, dst in ((0, q_pad), (1, k_pad)):
                    src = qkv_row[:, 768 * sec : 768 * (sec + 1)].rearrange(
                        "p (cb ci2) -> p cb ci2", ci2=64
                    )
                    dst_ap = bass.AP(
                        tensor=dst.tensor,
                        offset=dst.offset + r * 128 * 12 * 128,
                        ap=[[12 * 128, 128], [128, 12], [1, 64]],
                    )
                    nc.scalar.dma_start(out=dst_ap, in_=src)
                nc.scalar.dma_start(
                    out=v_scr[128 * r : 128 * (r + 1), :], in_=qkv_row[:, 1536:2304]
                )

        # Collapse the phase-1 scratch-write -> phase-2 read dependencies
        # into one sync point (HWDGE DMAs only have ~2 wait slots).
        tc.strict_bb_all_engine_barrier()

        # ---------------- Phase 2: attention per head ----------------
        with tc.tile_pool(name="qkt", bufs=3) as qktp, tc.tile_pool(
            name="vsb", bufs=3
        ) as vp, tc.tile_pool(name="ptp", bufs=3) as ptp, tc.tile_pool(
            name="stps", bufs=2, space="PSUM"
        ) as stps, tc.tile_pool(
            name="apsp", bufs=2, space="PSUM"
        ) as apsp, tc.tile_pool(
            name="bcps", bufs=2, space="PSUM"
        ) as bcp, tc.tile_pool(name="nrm", bufs=2) as nrmp:
            for h in range(3):
                QT = qktp.tile([128, 2048], bf16, name="QT", tag="QT")
                nc.sync.dma_start_transpose(QT, q_pad[2048 * h : 2048 * (h + 1), :])
                nc.gpsimd.memset(QT[64:128, :], 0.0)
                nc.gpsimd.memset(QT[64:65, :], 1.0)
                KT = qktp.tile([128, 2048], bf16, name="KT", tag="KT")
                nc.sync.dma_start_transpose(KT, k_pad[2048 * h : 2048 * (h + 1), :])
                nc.gpsimd.memset(KT[64:128, :], 0.0)
                nc.sync.dma_start(out=KT[64:65, :], in_=amask[0:1, :])
                v_sb = vp.tile([128, 16, 65], bf16, name="v_sb", tag="v_sb")
                src_v = bass.AP(
                    tensor=v_scr.tensor,
                    offset=v_scr.offset + h * 131072,
                    ap=[[64, 128], [8192, 16], [1, 64]],
                )
                nc.sync.dma_start(out=v_sb[:, :, 0:64], in_=src_v)
                nc.gpsimd.memset(v_sb[:, :, 64:65], 1.0)

                for qb in range(4):
                    nkt = 4 * (qb + 1)
                    a_ps = apsp.tile([65, 512], f32, name="a_ps", tag="a_ps")
                    for g2 in range(nkt // 2):
                        st = stps.tile([128, 1024], f32, name="st", tag="st")
                        for t2 in range(2):
                            kt2 = 2 * g2 + t2
                            nc.tensor.matmul(
                                st[:, 512 * t2 : 512 * (t2 + 1)],
                                KT[:, 128 * kt2 : 128 * (kt2 + 1)],
                                QT[:, 512 * qb : 512 * (qb + 1)],
                                start=True,
                                stop=True,
                            )
                        pt = ptp.tile([128, 1024], bf16, name="pt", tag="pt")
                        nc.scalar.activation(out=pt, in_=st, func=AF.Exp, scale=0.125)
                        for t2 in range(2):
                            kt2 = 2 * g2 + t2
                            o = 128 * kt2 - 512 * qb
                            if o >= 0:
                                # keep (q - k - o >= 0), zero the rest
                                nc.gpsimd.affine_select(
                                    out=pt[:, 512 * t2 : 512 * (t2 + 1)],
                                    in_=pt[:, 512 * t2 : 512 * (t2 + 1)],
                                    pattern=[[1, 512]],
                                    base=-o,
                                    channel_multiplier=-1,
                                    compare_op=ALU.is_ge,
                                    fill=0.0,
                                )
                        for t2 in range(2):
                            kt2 = 2 * g2 + t2
                            nc.tensor.matmul(
                                a_ps,
                                v_sb[:, kt2, :],
                                pt[:, 512 * t2 : 512 * (t2 + 1)],
                                start=(kt2 == 0),
                                stop=(kt2 == nkt - 1),
                                skip_group_check=True,
                            )
                    recip = nrmp.tile([1, 512], f32, name="recip", tag="recip")
                    nc.vector.reciprocal(recip, a_ps[64:65, :])
                    bc = bcp.tile([64, 512], f32, name="bc", tag="bc")
                    nc.tensor.matmul(
                        bc, ones_col, recip, start=True, stop=True,
                        skip_group_check=True,
                    )
                    tmp = nrmp.tile([64, 512], f32, name="tmp", tag="tmp")
                    nc.vector.tensor_copy(out=tmp, in_=a_ps[0:64, :])
                    nc.vector.tensor_mul(
                        out=AT[h][:, 512 * qb : 512 * (qb + 1)], in0=tmp, in1=bc
                    )

        # ---------------- Phase 3: output projection ----------------
        with tc.tile_pool(name="p3a", bufs=2, space="PSUM") as p3a, tc.tile_pool(
            name="p3b", bufs=2, space="PSUM"
        ) as p3b, tc.tile_pool(name="outp", bufs=3) as outp:
            for qt2 in range(16):
                ps_a = p3a.tile([128, 512], f32, name="ps_a", tag="ps_a")
                ps_b = p3b.tile([128, 256], f32, name="ps_b", tag="ps_b")
                for h in range(3):
                    at_sl = AT[h][:, 128 * qt2 : 128 * (qt2 + 1)]
                    nc.tensor.matmul(
                        ps_a,
                        at_sl,
                        wp_sb[h][:, 0:512],
                        start=(h == 0),
                        stop=(h == 2),
                        skip_group_check=True,
                    )
                    nc.tensor.matmul(
                        ps_b,
                        at_sl,
                        wp_sb[h][:, 512:768],
                        start=(h == 0),
                        stop=(h == 2),
                        skip_group_check=True,
                    )
                out_sb = outp.tile([128, 768], f32, name="out_sb", tag="out_sb")
                nc.scalar.copy(out=out_sb[:, 0:512], in_=ps_a)
                nc.scalar.copy(out=out_sb[:, 512:768], in_=ps_b)
                nc.scalar.dma_start(
                    out=out[128 * qt2 : 128 * (qt2 + 1), :], in_=out_sb
                )

    nc.compile()
    return nc


def kernel(hidden_states, attention_mask, w_attn, b_attn, w_proj, b_proj):
    global _CACHED_NC

    hs = np.asarray(hidden_states, dtype=np.float32)
    am = np.asarray(attention_mask, dtype=np.float32)
    wa = np.asarray(w_attn, dtype=np.float32)
    ba = np.asarray(b_attn, dtype=np.float32)
    wpj = np.asarray(w_proj, dtype=np.float32)
    bp = np.asarray(b_proj, dtype=np.float32)

    w_aug = np.zeros((896, 2304), np.float32)
    w_aug[0:768] = wa
    w_aug[768] = ba
    w_aug = w_aug.astype(BF16)

    in_maps = []
    for core in range(8):
        b, g = core // 4, core % 4
        xt_aug = np.zeros((896, 512), np.float32)
        xt_aug[0:768] = hs[b, 512 * g : 512 * (g + 1), :].T
        xt_aug[768] = 1.0
        in_maps.append(
            {
                "xt": np.ascontiguousarray(xt_aug).astype(BF16),
                "wqkv": w_aug,
                "wp": np.ascontiguousarray(wpj[192 * g : 192 * (g + 1), :]).astype(
                    BF16
                ),
                "amask": np.ascontiguousarray(
                    (8.0 * am[b, 0, 0, :]).reshape(1, 2048)
                ).astype(BF16),
            }
        )

    if _CACHED_NC is None:
        _CACHED_NC = _build_program()

    from concourse.bass_utils import run_bass_kernel_spmd

    res = run_bass_kernel_spmd(_CACHED_NC, in_maps, core_ids=list(range(8)))
    global _LAST_RESULTS
    _LAST_RESULTS = res
    outs = [r["out"] for r in res.results]

    result = np.zeros((2, 2048, 768), np.float32)
    for b in range(2):
        result[b] = (
            outs[4 * b]
            + outs[4 * b + 1]
            + outs[4 * b + 2]
            + outs[4 * b + 3]
            + bp[None, :]
        )
    return result


# revision 14
# speedup vs baseline: 1.0555x; 1.0555x over previous
"""Trainium2 Bass kernel for GPT2-style attention (nn_Attention_71030169141804).

Computation (faithful to the reference's direct-reshape head split):
  qkv = hidden @ w_attn + b_attn ; q,k,v = split(qkv)
  q/k/v heads = flat-memory reshape (bs, 12, 2048, 64)   <- NOT the standard
     split-heads transpose: head h covers a contiguous 131072-element block of
     the flattened per-batch (seq*768) q/k/v memory, i.e. heads {3g,3g+1,3g+2}
     derive exactly from hidden rows [512g, 512(g+1)).
  causal softmax(qk/8 + mask), @v, merge heads, @ w_proj + b_proj.

Sharding: 8 cores = (batch b in 0..1) x (row-group g in 0..3). Each core:
  - QKV-projects its 512 hidden rows (bias folded in via augmented ones-row),
  - runs attention for its 3 heads (full 2048 seq each),
  - computes the partial output projection with its 192 w_proj rows.
Host sums the 4 partials per batch and adds b_proj. No collectives.

Device dataflow per core (all matmuls bf16, fp32 accumulation):
  P1: qkv tiles = xt_aug.T @ w_aug -> psum -> sbuf bf16 -> DRAM scratch
      (q/k written into a per-head zero-padded [2048,128] layout).
  P2/P3 interleaved by q-block: per head, QT/KT via DMA-transpose
      ([2048,128] -> [128,2048]); row 64 carries the attention-mask trick
      (KT row64 = 8*mask, QT row64 = 1) so S^T = KT.T@QT includes the
      additive mask exactly; exp fused with the 1/sqrt(64) scale on ACT;
      causal masking via affine_select on diagonal tiles; PV with V
      augmented by a ones column so psum row 64 accumulates the softmax
      denominators; normalization uses exp(-ln(denom)) on ACT plus a
      stride-0 DMA partition-broadcast; the output projection for each
      q-block runs as soon as its three heads' A^T slices are normalized.
"""

import numpy as np
import ml_dtypes

BF16 = ml_dtypes.bfloat16

_CACHED_NC = None
_LAST_RESULTS = None


def _build_program():
    from contextlib import ExitStack

    import concourse.bass as bass
    import concourse.tile as tile
    from concourse import bacc, mybir

    dt = mybir.dt
    f32 = dt.float32
    bf16 = dt.bfloat16
    AF = mybir.ActivationFunctionType
    ALU = mybir.AluOpType

    nc = bacc.Bacc("TRN2")
    xt_d = nc.dram_tensor("xt", [896, 512], bf16, kind="ExternalInput")
    wqkv_d = nc.dram_tensor("wqkv", [896, 2304], bf16, kind="ExternalInput")
    wp_d = nc.dram_tensor("wp", [192, 768], bf16, kind="ExternalInput")
    amask_d = nc.dram_tensor("amask", [1, 2048], bf16, kind="ExternalInput")
    out_d = nc.dram_tensor("out", [2048, 768], f32, kind="ExternalOutput")

    xt, wqkv, wp, amask, out = (
        xt_d.ap(),
        wqkv_d.ap(),
        wp_d.ap(),
        amask_d.ap(),
        out_d.ap(),
    )

    with tile.TileContext(nc) as tc, ExitStack() as ctx:
        singles = ctx.enter_context(tc.tile_pool(name="singles", bufs=1))
        dram = ctx.enter_context(tc.tile_pool(name="dram", bufs=1, space="DRAM"))

        # xt before w: the first matmul group needs xt_sb[*] + w_sb[0], and
        # the sync HWDGE ring is FIFO - don't make it drain 4MB of W first.
        xt_sb = []
        for kt in range(7):
            t = singles.tile([128, 512], bf16, name=f"xt_sb{kt}", tag=f"xt_sb{kt}")
            nc.sync.dma_start(out=t, in_=xt[128 * kt : 128 * (kt + 1), :])
            xt_sb.append(t)
        wp_sb = []
        for h in range(3):
            t = singles.tile([64, 768], bf16, name=f"wp_sb{h}", tag=f"wp_sb{h}")
            nc.sync.dma_start(out=t, in_=wp[64 * h : 64 * (h + 1), :])
            wp_sb.append(t)
        w_sb = []
        for kt in range(7):
            t = singles.tile([128, 2304], bf16, name=f"w_sb{kt}", tag=f"w_sb{kt}")
            nc.sync.dma_start(out=t, in_=wqkv[128 * kt : 128 * (kt + 1), :])
            w_sb.append(t)
        AT = []
        for h in range(3):
            t = singles.tile([64, 2048], bf16, name=f"at{h}", tag=f"at{h}")
            AT.append(t)

        q_pad = dram.tile([6144, 128], bf16, name="q_pad", tag="q_pad")
        k_pad = dram.tile([6144, 128], bf16, name="k_pad", tag="k_pad")
        v_scr = dram.tile([512, 768], bf16, name="v_scr", tag="v_scr")

        # One-time zero fill of the pad columns (64:128) of q_pad/k_pad so the
        # DMA-transpose brings in zeros for partitions 64..127 (the contraction
        # over those rows then contributes nothing). Same scalar HWDGE ring as
        # the scatter writes -> readers need a single ring wait.
        zeros_sb = singles.tile([128, 3072], bf16, name="zeros_sb", tag="zeros_sb")
        nc.vector.memset(zeros_sb, 0.0)
        for dst in (q_pad, k_pad):
            pad_ap = bass.AP(
                tensor=dst.tensor,
                offset=dst.offset + 64,
                ap=[[48 * 128, 128], [128, 48], [1, 64]],
            )
            nc.scalar.dma_start(
                out=pad_ap, in_=zeros_sb.rearrange("p (a c) -> p a c", c=64)
            )

        # ---------------- Phase 1: QKV projection ----------------
        CCH = [(0, 512), (512, 512), (1024, 512), (1536, 512), (2048, 256)]
        with tc.tile_pool(name="p1ps", bufs=3, space="PSUM") as p1ps, tc.tile_pool(
            name="qkvrow", bufs=4
        ) as qkvp:
            for r in range(4):
                qkv_row = qkvp.tile([128, 2304], bf16, name="qkv_row", tag="qkv_row")
                for ci, (c0, cw) in enumerate(CCH):
                    ps = p1ps.tile([128, 512], f32, name="p1t", tag="p1t")
                    for kt in range(7):
                        nc.tensor.matmul(
                            ps[:, :cw],
                            xt_sb[kt][:, 128 * r : 128 * (r + 1)],
                            w_sb[kt][:, c0 : c0 + cw],
                            start=(kt == 0),
                            stop=(kt == 6),
                        )
                    nc.vector.tensor_copy(out=qkv_row[:, c0 : c0 + cw], in_=ps[:, :cw])
                # q/k sections -> padded per-head layout: flat element
                # (128r+p)*768 + 64*cb + ci2 lands at row 12*(128r+p)+cb,
                # col ci2 of the [6144,128] scratch.
                for sec, dst in ((0, q_pad), (1, k_pad)):
                    src = qkv_row[:, 768 * sec : 768 * (sec + 1)].rearrange(
                        "p (cb ci2) -> p cb ci2", ci2=64
                    )
                    dst_ap = bass.AP(
                        tensor=dst.tensor,
                        offset=dst.offset + r * 128 * 12 * 128,
                        ap=[[12 * 128, 128], [128, 12], [1, 64]],
                    )
                    nc.scalar.dma_start(out=dst_ap, in_=src)
                nc.scalar.dma_start(
                    out=v_scr[128 * r : 128 * (r + 1), :], in_=qkv_row[:, 1536:2304]
                )

        # ---------------- Phases 2+3: attention + projection ----------------
        with tc.tile_pool(name="qkt", bufs=3) as qktp, tc.tile_pool(
            name="vsb", bufs=3
        ) as vp, tc.tile_pool(name="ptp", bufs=3) as ptp, tc.tile_pool(
            name="nrm", bufs=3
        ) as nrmp, tc.tile_pool(
            name="outp", bufs=3
        ) as outp, tc.tile_pool(
            name="stps", bufs=2, space="PSUM"
        ) as stps, tc.tile_pool(
            name="apsp", bufs=2, space="PSUM"
        ) as apsp, tc.tile_pool(name="prps", bufs=1, space="PSUM") as prps:
            QT, KT, VS = [], [], []
            for h in range(3):
                qt_t = qktp.tile([128, 2048], bf16, name=f"QT{h}", tag="QT")
                nc.sync.dma_start_transpose(qt_t, q_pad[2048 * h : 2048 * (h + 1), :])
                nc.gpsimd.memset(qt_t[64:65, :], 1.0)
                kt_t = qktp.tile([128, 2048], bf16, name=f"KT{h}", tag="KT")
                nc.scalar.dma_start_transpose(kt_t, k_pad[2048 * h : 2048 * (h + 1), :])
                nc.sync.dma_start(out=kt_t[64:65, :], in_=amask[0:1, :])
                v_sb = vp.tile([128, 16, 65], bf16, name=f"v_sb{h}", tag="v_sb")
                src_v = bass.AP(
                    tensor=v_scr.tensor,
                    offset=v_scr.offset + h * 131072,
                    ap=[[64, 128], [8192, 16], [1, 64]],
                )
                nc.scalar.dma_start(out=v_sb[:, :, 0:64], in_=src_v)
                nc.gpsimd.memset(v_sb[:, :, 64:65], 1.0)
                QT.append(qt_t)
                KT.append(kt_t)
                VS.append(v_sb)

            for qb in range(4):
                nkt = 4 * (qb + 1)
                for h in range(3):
                    a_ps = apsp.tile([65, 512], f32, name="a_ps", tag="a_ps")
                    for g2 in range(nkt // 2):
                        st = stps.tile([128, 1024], f32, name="st", tag="st")
                        for t2 in range(2):
                            kt2 = 2 * g2 + t2
                            nc.tensor.matmul(
                                st[:, 512 * t2 : 512 * (t2 + 1)],
                                KT[h][:, 128 * kt2 : 128 * (kt2 + 1)],
                                QT[h][:, 512 * qb : 512 * (qb + 1)],
                                start=True,
                                stop=True,
                            )
                        pt = ptp.tile([128, 1024], bf16, name="pt", tag="pt")
                        nc.scalar.activation(out=pt, in_=st, func=AF.Exp, scale=0.125)
                        for t2 in range(2):
                            kt2 = 2 * g2 + t2
                            o = 128 * kt2 - 512 * qb
                            if o >= 0:
                                # keep (q - k - o >= 0), zero the rest
                                nc.gpsimd.affine_select(
                                    out=pt[:, 512 * t2 : 512 * (t2 + 1)],
                                    in_=pt[:, 512 * t2 : 512 * (t2 + 1)],
                                    pattern=[[1, 512]],
                                    base=-o,
                                    channel_multiplier=-1,
                                    compare_op=ALU.is_ge,
                                    fill=0.0,
                                )
                        for t2 in range(2):
                            kt2 = 2 * g2 + t2
                            nc.tensor.matmul(
                                a_ps,
                                VS[h][:, kt2, :],
                                pt[:, 512 * t2 : 512 * (t2 + 1)],
                                start=(kt2 == 0),
                                stop=(kt2 == nkt - 1),
                                skip_group_check=True,
                            )
                    # normalize: 1/denom via exp(-ln(d)) on ACT (vector
                    # reciprocal on a [1,512] AP is single-lane and slow),
                    # broadcast to 64 partitions with a stride-0 DMA.
                    lg = nrmp.tile([1, 512], f32, name="lg", tag="lg")
                    nc.scalar.activation(out=lg, in_=a_ps[64:65, :], func=AF.Ln)
                    rc = nrmp.tile([1, 512], f32, name="rc", tag="rc")
                    nc.scalar.activation(out=rc, in_=lg, func=AF.Exp, scale=-1.0)
                    bc = nrmp.tile([64, 512], f32, name="bc", tag="bc")
                    nc.gpsimd.partition_broadcast(bc, rc, channels=64)
                    nc.vector.tensor_mul(
                        out=AT[h][:, 512 * qb : 512 * (qb + 1)],
                        in0=a_ps[0:64, :],
                        in1=bc,
                    )
                # output projection for this q-range
                for qt2 in range(4 * qb, 4 * qb + 4):
                    pr = prps.tile([128, 768], f32, name="pr", tag="pr")
                    for h in range(3):
                        at_sl = AT[h][:, 128 * qt2 : 128 * (qt2 + 1)]
                        nc.tensor.matmul(
                            pr[:, 0:512],
                            at_sl,
                            wp_sb[h][:, 0:512],
                            start=(h == 0),
                            stop=(h == 2),
                            skip_group_check=True,
                        )
                        nc.tensor.matmul(
                            pr[:, 512:768],
                            at_sl,
                            wp_sb[h][:, 512:768],
                            start=(h == 0),
                            stop=(h == 2),
                            skip_group_check=True,
                        )
                    out_sb = outp.tile([128, 768], f32, name="out_sb", tag="out_sb")
                    nc.vector.tensor_copy(out=out_sb, in_=pr)
                    nc.sync.dma_start(
                        out=out[128 * qt2 : 128 * (qt2 + 1), :], in_=out_sb
                    )

    nc.compile()
    return nc


def kernel(hidden_states, attention_mask, w_attn, b_attn, w_proj, b_proj):
    global _CACHED_NC, _LAST_RESULTS

    hs = np.asarray(hidden_states, dtype=np.float32)
    am = np.asarray(attention_mask, dtype=np.float32)
    wa = np.asarray(w_attn, dtype=np.float32)
    ba = np.asarray(b_attn, dtype=np.float32)
    wpj = np.asarray(w_proj, dtype=np.float32)
    bp = np.asarray(b_proj, dtype=np.float32)

    w_aug = np.zeros((896, 2304), np.float32)
    w_aug[0:768] = wa
    w_aug[768] = ba
    w_aug = w_aug.astype(BF16)

    in_maps = []
    for core in range(8):
        b, g = core // 4, core % 4
        xt_aug = np.zeros((896, 512), np.float32)
        xt_aug[0:768] = hs[b, 512 * g : 512 * (g + 1), :].T
        xt_aug[768] = 1.0
        in_maps.append(
            {
                "xt": np.ascontiguousarray(xt_aug).astype(BF16),
                "wqkv": w_aug,
                "wp": np.ascontiguousarray(wpj[192 * g : 192 * (g + 1), :]).astype(
                    BF16
                ),
                "amask": np.ascontiguousarray(
                    (8.0 * am[b, 0, 0, :]).reshape(1, 2048)
                ).astype(BF16),
            }
        )

    if _CACHED_NC is None:
        _CACHED_NC = _build_program()

    from concourse.bass_utils import run_bass_kernel_spmd

    res = run_bass_kernel_spmd(_CACHED_NC, in_maps, core_ids=list(range(8)))
    _LAST_RESULTS = res
    outs = [r["out"] for r in res.results]

    result = np.zeros((2, 2048, 768), np.float32)
    for b in range(2):
        result[b] = (
            outs[4 * b]
            + outs[4 * b + 1]
            + outs[4 * b + 2]
            + outs[4 * b + 3]
            + bp[None, :]
        )
    return result
